# revision 41
# baseline (speedup 1.0000x reference)
"""DiffuCoder attention (non-causal, GQA) on 8 trn2 NeuronCores.

Sharding: Megatron-style head parallelism. Core c owns query heads
{2c, 2c+1} and KV head c//2 (column-parallel Wq/Wk/Wv), plus the
matching 256 rows of Wo (row-parallel). Each core writes a partial
out^T [H, S]; the host sums the 8 partials and transposes back.

Device-side layout tricks (prepared on host):
  - hs is fed pre-transposed (hsT [hidden, seq]) so every projection
    matmul has its natural operand orientation and the kernel needs no
    on-device transposes at all.
  - Q/K feature order is permuted per head to [ev0..15 | od0..15 |
    ev16..31 | od16..31 | pass 64..127] so the interleaved-RoPE pair
    swap is a single 32-partition-quadrant stream_shuffle on DVE.
  - softmax scale 1/sqrt(128) is baked into Wq.
  - scores are computed transposed (s_k on partitions) so the O matmul
    (attn @ v) needs no transpose either; softmax denominators are
    split across engines: exp tiles 0..7 are summed by a 3-level
    bf16 halving tree on DVE (fast-mode ops, fp32 final level) finished
    with a gpsimd partition_all_reduce,
    tiles 8..15 are summed on the PE via a ones-stationary matmul
    (whose result lands pre-broadcast across partitions). The 1/sum
    normalization is folded into the PSUM-evacuation multiply (legal
    because a per-token scale commutes with Wo).
All matmuls run in bf16 with fp32 PSUM accumulation; softmax skips the
max-subtraction (scores here are O(1), far inside fp32 exp range). The
partial out^T is written in bf16 (halves output HBM traffic; the host
sums the 8 partials in float64, so only one extra rounding step).

Perf notes (measured on this trn2 via the R2-R1 reps-loop delta):
  - effective PE matmul cost ~= (moving_rows + 128 ldweights rows)/1.9GHz;
    the ~54ns/matmul issue+LW overhead makes instruction count matter as
    much as row count, so V is projected K-style ([d,seq], 512-wide
    moving, 64 matmuls instead of 256) and transposed into its [seq,d]
    O-matmul-stationary layout by a single XBAR dma_start_transpose.
  - exp evacuations read [128,1024] PSUM (2 banks) halving ACT instrs.
  - phase A consumes hsT in DMA-arrival order (K/V/Q interleaved per
    chunk); weight DMAs are ordered so the only end-of-iteration WAR
    (wo) sits last on the ACT HWDGE queue and cannot head-of-line-block
    the next iteration's hsT prefetch on the SP queue.
  - outproj PSUM tiles alternate between the psAW and psO rings so the
    2-deep rings get ~2 matmuls of evacuation-drain time each.
  - full-DVE softmax denominators and gpsimd pair-sums were measured
    SLOWER (254-353us) than the hybrid DVE-tree + ones-matmul split, and
    merging both heads' score rounds into one long stream with outproj
    matmuls woven in as filler was also slower (249-254us) — the 8-round
    blocks with O/ones matmuls between them reset the ACT exp lag.
  - outproj evacuations: ACT takes the FIRST 8 mt, DVE the last 8 —
    ACT's in-order queue is then drained when the next attention
    block's exps arrive (the old alternating split left ACT copies
    queued ahead of the exps, stalling the score-round PSUM ring:
    fixing this was worth ~10us). Packing outproj PSUM into the idle
    psS tiles instead measured ~24us slower (couples score rounds to
    outproj drains via the pss ring).
Measured: ~236-239 us/pass at short bursts (33/161 reps), ~236-247 us
sustained (65/577 reps, chip-window dependent; best 235771 ns), rel err
~5.4e-3. Baseline carried into this session measured ~287-293 us on the
same chip. Also tried and reverted: splitting the tail outproj
evacuations into parallel DVE+ACT halves to free PSUM banks faster
(+32 instructions) measured ~8 us SLOWER — per-instruction overhead
beats ring-stall savings every time on this machine.
"""

import sys

sys.path.insert(0, "/opt/trn_rl_repo")

import numpy as np
import ml_dtypes

import concourse.bacc as bacc
import concourse.mybir as mybir
import concourse.tile as tile
import concourse.bass_isa as bass_isa
from concourse import bass_utils

BF16 = ml_dtypes.bfloat16

S = 2048        # sequence length
H = 2048        # hidden
D = 128         # head dim
N_HEADS = 16
N_KV = 4
N_CORES = 8
QH = N_HEADS // N_CORES     # q heads per core = 2
THETA = 10000.0
CH = 512                    # seq chunk for matmul free dim
N_CH = S // CH              # 4
N_HT = H // 128             # 16 hidden tiles
N_SK = S // 128             # 16 key tiles

_SHUF_MASK = list(range(16, 32)) + list(range(16))


def _rope_perm():
    """Per-head feature permutation new_row -> orig_feature."""
    p = np.empty(128, dtype=np.int64)
    r = np.arange(16)
    p[0:16] = 2 * r
    p[16:32] = 2 * r + 1
    p[32:48] = 2 * (r + 16)
    p[48:64] = 2 * (r + 16) + 1
    p[64:128] = np.arange(64, 128)
    return p


def _rope_tables():
    """tabC/tabS [64, S] fp32 matching the permuted layout."""
    inv_freq = 1.0 / THETA ** (np.arange(0, 64, 2, dtype=np.float64) / 64.0)  # [32]
    pos = np.arange(S, dtype=np.float64)
    ang = pos[None, :] * inv_freq[:, None]          # [32, S]
    cos, sin = np.cos(ang), np.sin(ang)
    fi = np.concatenate([np.arange(16), np.arange(16),
                         np.arange(16, 32), np.arange(16, 32)])
    sign = np.where((np.arange(64) // 16) % 2 == 0, -1.0, 1.0)
    tabC = cos[fi, :].astype(np.float32)
    tabS = (sign[:, None] * sin[fi, :]).astype(np.float32)
    return tabC, tabS


def _build(reps=1):
    """Build + bacc-compile the per-core kernel module."""
    dt = mybir.dt
    nc = bacc.Bacc("TRN2", target_bir_lowering=False, debug=False)

    hsT_d = nc.dram_tensor("hsT", [H, S], dt.bfloat16, kind="ExternalInput")
    wq_d = nc.dram_tensor("wq", [H, QH * D], dt.bfloat16, kind="ExternalInput")
    wk_d = nc.dram_tensor("wk", [H, D], dt.bfloat16, kind="ExternalInput")
    wv_d = nc.dram_tensor("wv", [H, D], dt.bfloat16, kind="ExternalInput")
    wo_d = nc.dram_tensor("wo", [QH * D, H], dt.bfloat16, kind="ExternalInput")
    tabc_d = nc.dram_tensor("tabc", [64, S], dt.float32, kind="ExternalInput")
    tabs_d = nc.dram_tensor("tabs", [64, S], dt.float32, kind="ExternalInput")
    outT_d = nc.dram_tensor("outT", [H, S], dt.bfloat16, kind="ExternalOutput")

    with tile.TileContext(nc) as tc:
        from contextlib import ExitStack

        with ExitStack() as ctx:
            const = ctx.enter_context(tc.tile_pool(name="const", bufs=1))
            persist = ctx.enter_context(tc.tile_pool(name="persist", bufs=1))
            expp = ctx.enter_context(tc.tile_pool(name="expp", bufs=2))
            rope = ctx.enter_context(tc.tile_pool(name="rope", bufs=2))
            sums = ctx.enter_context(tc.tile_pool(name="sums", bufs=1))
            ostage = ctx.enter_context(tc.tile_pool(name="ostage", bufs=4))
            psAW = ctx.enter_context(tc.tile_pool(name="psAW", bufs=2, space="PSUM"))
            psS = ctx.enter_context(tc.tile_pool(name="psS", bufs=2, space="PSUM"))
            psO = ctx.enter_context(tc.tile_pool(name="psO", bufs=2, space="PSUM"))

            def body(_iv=None):
                # ---- load weights/tables (small, needed first) ----
                # wk/wv/wq before hsT; wo last and on the ACT HWDGE queue:
                # its WAR (last outproj read) only clears at end-of-iteration,
                # and it must not head-of-line-block the next iteration's hsT
                # prefetch on the SP queue.
                wk_sb = const.tile([128, N_HT * D], dt.bfloat16, tag="wk")
                nc.sync.dma_start(
                    wk_sb[:].rearrange("p (t f) -> p t f", t=N_HT),
                    wk_d[:].rearrange("(t p) f -> p t f", p=128))
                wv_sb = const.tile([128, N_HT * D], dt.bfloat16, tag="wv")
                nc.sync.dma_start(
                    wv_sb[:].rearrange("p (t f) -> p t f", t=N_HT),
                    wv_d[:].rearrange("(t p) f -> p t f", p=128))
                wq_sb = const.tile([128, N_HT * QH * D], dt.bfloat16, tag="wq")
                nc.sync.dma_start(
                    wq_sb[:].rearrange("p (t f) -> p t f", t=N_HT),
                    wq_d[:].rearrange("(t p) f -> p t f", p=128))

                tabC = const.tile([64, S], dt.float32, tag="tabC")
                nc.sync.dma_start(tabC[:], tabc_d[:])
                tabS = const.tile([64, S], dt.float32, tag="tabS")
                nc.sync.dma_start(tabS[:], tabs_d[:])

                # hsT in quarter-width pieces, chunk-major, so the first
                # projection matmuls (which contract over ALL h-tiles but only
                # read 512 seq columns) can start after ~2MB instead of 8MB
                hsT_sb = const.tile([128, N_HT * S], dt.bfloat16, tag="hsT")
                for q in range(N_CH):
                    c0, c1 = q * CH, (q + 1) * CH
                    for t in range(N_HT):
                        nc.sync.dma_start(
                            hsT_sb[:, t * S + c0: t * S + c1],
                            hsT_d[t * 128:(t + 1) * 128, c0:c1])

                wo_sb = const.tile([128, 2 * H], dt.bfloat16, tag="wo")
                nc.scalar.dma_start(
                    wo_sb[:].rearrange("p (t f) -> p t f", t=QH),
                    wo_d[:].rearrange("(t p) f -> p t f", p=128))

                ones_sb = const.tile([128, 128], dt.bfloat16, tag="ones")
                nc.gpsimd.memset(ones_sb[:], 1.0)


                qT = [persist.tile([128, S], dt.bfloat16, tag=f"qT{m}",
                                   name=f"qT{m}") for m in range(QH)]
                kT = persist.tile([128, S], dt.bfloat16, tag="kT")
                vT = persist.tile([128, S], dt.bfloat16, tag="vT")
                v_sb = persist.tile([128, N_SK * D], dt.bfloat16, tag="v")
                oT = [persist.tile([128, S], dt.bfloat16, tag=f"oT{m}",
                                   name=f"oT{m}") for m in range(QH)]

                def rope_evac(ps, dst, ch):
                    sl = slice(ch * CH, (ch + 1) * CH)
                    xs = rope.tile([64, CH], dt.float32, tag="xs")
                    nc.vector.stream_shuffle(xs[:], ps[0:64, :], _SHUF_MASK)
                    ra = rope.tile([64, CH], dt.float32, tag="ra")
                    nc.vector.tensor_mul(ra[:], ps[0:64, :], tabC[:, sl])
                    rb = rope.tile([64, CH], dt.float32, tag="rb")
                    nc.vector.tensor_mul(rb[:], xs[:], tabS[:, sl])
                    nc.vector.tensor_add(dst[0:64, sl], ra[:], rb[:])
                    # last chunk's pass-through copies go to DVE so the ACT
                    # queue is drained when phase B's first exps arrive
                    if ch == N_CH - 1:
                        nc.vector.tensor_copy(dst[64:128, sl], ps[64:128, :])
                    else:
                        nc.scalar.copy(dst[64:128, sl], ps[64:128, :])

                # ---- phase A: projections ----
                # K/V/Q interleaved per seq chunk, in DMA-arrival order, so
                # the PE never waits for the hsT second half. V is computed in
                # the same [d, seq] orientation as K (512-wide moving), then
                # one XBAR DMA-transpose gives the [seq, d] tile layout the O
                # matmuls need as stationary.
                for ch in range(N_CH):
                    ps = psAW.tile([128, CH], dt.float32, tag="ps")
                    for h in range(N_HT):
                        nc.tensor.matmul(
                            ps[:], wk_sb[:, h * D:(h + 1) * D],
                            hsT_sb[:, h * S + ch * CH: h * S + (ch + 1) * CH],
                            start=(h == 0), stop=(h == N_HT - 1))
                    rope_evac(ps, kT, ch)

                    ps = psAW.tile([128, CH], dt.float32, tag="ps", name="psv")
                    for h in range(N_HT):
                        nc.tensor.matmul(
                            ps[:], wv_sb[:, h * D:(h + 1) * D],
                            hsT_sb[:, h * S + ch * CH: h * S + (ch + 1) * CH],
                            start=(h == 0), stop=(h == N_HT - 1))
                    nc.vector.tensor_copy(vT[:, ch * CH:(ch + 1) * CH], ps[:])

                    for m in range(QH):
                        ps = psAW.tile([128, CH], dt.float32, tag="ps", name="psq")
                        for h in range(N_HT):
                            nc.tensor.matmul(
                                ps[:],
                                wq_sb[:, h * QH * D + m * D: h * QH * D + (m + 1) * D],
                                hsT_sb[:, h * S + ch * CH: h * S + (ch + 1) * CH],
                                start=(h == 0), stop=(h == N_HT - 1))
                        rope_evac(ps, qT[m], ch)

                nc.sync.dma_start_transpose(
                    v_sb[:].rearrange("p (t f) -> p t f", t=N_SK), vT[:])

                # ---- phase B: attention (per head, per sq chunk) ----
                exp_tiles = {}

                def attn_scores_round(m, ch, r):
                    sl = slice(ch * CH, (ch + 1) * CH)
                    if r == 0:
                        exp_tiles[m] = expp.tile([128, N_SK * CH], dt.bfloat16,
                                                 tag="expT", name=f"expT{m}_{ch}")
                    expT = exp_tiles[m]
                    # scores in [128, 2*CH] PSUM tiles (2 banks) so each exp
                    # reads 1024 columns -> half the ACT instruction count
                    pss = psS.tile([128, 2 * CH], dt.float32, tag="pss")
                    for half in range(2):
                        sk = 2 * r + half
                        nc.tensor.matmul(
                            pss[:, half * CH:(half + 1) * CH],
                            kT[:, sk * 128:(sk + 1) * 128],
                            qT[m][:, sl], start=True, stop=True)
                    nc.scalar.activation(expT[:, 2 * r * CH:(2 * r + 2) * CH],
                                         pss[:],
                                         mybir.ActivationFunctionType.Exp)

                def attn_tail(m, ch):
                    sl = slice(ch * CH, (ch + 1) * CH)
                    expT = exp_tiles[m]
                    po = psO.tile([128, CH], dt.float32, tag="po")
                    for sk in range(N_SK):
                        nc.tensor.matmul(po[:], v_sb[:, sk * D:(sk + 1) * D],
                                         expT[:, sk * CH:(sk + 1) * CH],
                                         start=(sk == 0), stop=(sk == N_SK - 1))
                    # first half of the denominators: 3-level halving tree
                    # over exp tiles 0..7 on DVE; bf16 intermediates keep the
                    # ops in the DVE fast mode, final level lands in fp32
                    c1 = sums.tile([128, 4 * CH], dt.bfloat16, tag="c1", bufs=1,
                                   name=f"c1_{m}_{ch}")
                    nc.vector.tensor_add(c1[:], expT[:, 0:4 * CH],
                                         expT[:, 4 * CH:8 * CH])
                    c2 = sums.tile([128, 2 * CH], dt.bfloat16, tag="c2", bufs=1,
                                   name=f"c2_{m}_{ch}")
                    nc.vector.tensor_add(c2[:], c1[:, 0:2 * CH], c1[:, 2 * CH:4 * CH])
                    acc = sums.tile([128, CH], dt.float32, tag="acc",
                                    name=f"acc{m}_{ch}")
                    nc.vector.tensor_add(acc[:], c2[:, 0:CH], c2[:, CH:2 * CH])
                    # other half of the softmax denominators on the PE:
                    # ones-stationary matmul over tiles 8..15 (broadcast
                    # across partitions), then combine with the DVE half
                    # (which still needs its partition all-reduce)
                    pu = psO.tile([128, CH], dt.float32, tag="po", name=f"pu{m}_{ch}")
                    for sk in range(8, N_SK):
                        nc.tensor.matmul(pu[:], ones_sb[:],
                                         expT[:, sk * CH:(sk + 1) * CH],
                                         start=(sk == 8), stop=(sk == N_SK - 1))
                    bc = sums.tile([128, CH], dt.float32, tag="bc",
                                   name=f"bc{m}_{ch}")
                    nc.gpsimd.partition_all_reduce(bc[:], acc[:], channels=128,
                                                   reduce_op=bass_isa.ReduceOp.add)
                    sm = sums.tile([128, CH], dt.float32, tag="sm",
                                   name=f"sm{m}_{ch}")
                    nc.vector.tensor_add(sm[:], bc[:], pu[:])
                    rc = sums.tile([128, CH], dt.float32, tag="rc",
                                   name=f"rc{m}_{ch}")
                    nc.vector.reciprocal(rc[:], sm[:])
                    nc.vector.tensor_mul(oT[m][:, sl], po[:], rc[:])

                # ---- phase C: output projection ----
                def outproj_mt(ch, mt, tail=False):
                    sl = slice(ch * CH, (ch + 1) * CH)
                    # pw stays on the psAW ring: borrowing the psO ring
                    # (alternation) or packing pairs into idle psS tiles both
                    # measured slower — the pss variant couples the next
                    # attention block's score rounds to outproj drains
                    pw = psAW.tile([128, CH], dt.float32, tag="ps", name="pw")
                    for k in range(QH):
                        nc.tensor.matmul(
                            pw[:], wo_sb[:, k * H + mt * 128: k * H + (mt + 1) * 128],
                            oT[k][:, sl], start=(k == 0), stop=(k == QH - 1))
                    st = ostage.tile([128, CH], dt.bfloat16, tag="st")
                    if tail:
                        # tail chunks (no exps follow): split the evacuation
                        # into parallel DVE+ACT halves so the PSUM bank frees
                        # in ~450ns instead of ~900 — the 2-deep ring's 674ns
                        # matmul spacing then covers it without stalling
                        nc.vector.tensor_copy(st[:, 0:CH // 2], pw[:, 0:CH // 2])
                        nc.scalar.copy(st[:, CH // 2:CH], pw[:, CH // 2:CH])
                    elif mt < N_HT // 2:
                        # interleaved chunks: ACT takes the FIRST half of the
                        # evacuations, DVE the second, so ACT's in-order queue
                        # is drained when the next attention block's exps
                        # arrive instead of stalling the score-round ring
                        nc.scalar.copy(st[:], pw[:])
                    else:
                        nc.vector.tensor_copy(st[:], pw[:])
                    nc.sync.dma_start(
                        outT_d[mt * 128:(mt + 1) * 128, sl], st[:])

                # pipeline: scores(both heads) -> [outproj(ch-2) as PE
                # filler while ACT finishes the exps] -> O matmuls.
                # Measured dead ends kept out of this schedule: merging both
                # heads' 16 score rounds into one stream and weaving the
                # outproj matmuls into it as fine-grained filler was 3-8us
                # SLOWER on HW (the 8-round blocks with O/ones matmuls
                # between them reset the exp lag; a long ACT-gated stream
                # accumulates it).
                def attn(m, ch):
                    for r in range(N_SK // 2):
                        attn_scores_round(m, ch, r)
                    attn_tail(m, ch)

                attn(0, 0); attn(1, 0)
                attn(0, 1); attn(1, 1)
                for mt in range(N_HT):
                    outproj_mt(0, mt)
                attn(0, 2); attn(1, 2)
                for mt in range(N_HT):
                    outproj_mt(1, mt)
                attn(0, 3); attn(1, 3)
                for mt in range(N_HT):
                    outproj_mt(2, mt)
                for mt in range(N_HT):
                    outproj_mt(3, mt)

            if reps == 1:
                body()
            else:
                hint = (mybir.EngineType.PE, mybir.EngineType.DVE,
                        mybir.EngineType.Activation, mybir.EngineType.SP,
                        mybir.EngineType.Pool)
                with tc.For_i(0, reps, 1, hint_engines=hint) as i:
                    body(i)

    nc.compile()
    return nc


def _shard_inputs(hidden_states, Wq, Wk, Wv, Wo):
    """Host-side sharding/permutation. Returns in_maps for 8 cores."""
    hs = np.asarray(hidden_states, dtype=np.float32).reshape(S, H)
    Wq = np.asarray(Wq, dtype=np.float32)
    Wk = np.asarray(Wk, dtype=np.float32)
    Wv = np.asarray(Wv, dtype=np.float32)
    Wo = np.asarray(Wo, dtype=np.float32)

    hsT = np.ascontiguousarray(hs.T).astype(BF16)
    perm = _rope_perm()
    tabC, tabS = _rope_tables()
    scale = 1.0 / np.sqrt(np.float32(D))

    in_maps = []
    for c in range(N_CORES):
        g = c // 2
        wq_c = np.empty((H, QH * D), dtype=np.float32)
        for m in range(QH):
            h = QH * c + m
            wq_c[:, m * D:(m + 1) * D] = Wq[:, h * D + perm] * scale
        wk_c = Wk[:, g * D + perm]
        wv_c = np.ascontiguousarray(Wv[:, g * D:(g + 1) * D])
        wo_c = np.ascontiguousarray(Wo[QH * D * c: QH * D * (c + 1), :])
        in_maps.append({
            "hsT": hsT,
            "wq": wq_c.astype(BF16),
            "wk": np.ascontiguousarray(wk_c).astype(BF16),
            "wv": wv_c.astype(BF16),
            "wo": wo_c.astype(BF16),
            "tabc": tabC,
            "tabs": tabS,
        })
    return in_maps


_NC_CACHE = {}


def _get_nc(reps=1):
    if reps not in _NC_CACHE:
        _NC_CACHE[reps] = _build(reps)
    return _NC_CACHE[reps]


def kernel(hidden_states, Wq, Wk, Wv, Wo):
    nc = _get_nc(1)
    in_maps = _shard_inputs(hidden_states, Wq, Wk, Wv, Wo)
    res = bass_utils.run_bass_kernel_spmd(nc, in_maps, core_ids=list(range(N_CORES)))
    acc = np.zeros((H, S), dtype=np.float64)
    for c in range(N_CORES):
        acc += res.results[c]["outT"].astype(np.float64)
    out = acc.T.astype(np.float32).reshape(1, S, H)
    return out

